# revision 14
# baseline (speedup 1.0000x reference)
"""Trainium2 Bass kernel for nn_CornerActivationB.

Math: the reference expands a binary corner table [G, 4, D] to a ternary
grid [G, 9, D] via midpoint averaging, then does piecewise-bilinear
interpolation on the 3x3 grid. Midpoints are exact averages, so the
piecewise-bilinear interpolant of those samples IS the bilinear function
of the 4 binary corners:

    out[b, g, d] = c0[g,d] + u0*c1[g,d] + u1*c2[g,d] + u0*u1*c3[g,d]

with u = clip(x, -1, 1) and c* fixed +-0.25-multiples of corner sums.

v3 (transpose-free): the baseline was DMA-limited; v2 (uint8 out, bf16
in) fixed that but left the kernel dependency-bound: 123 PE transposes
plus qT PSUM->SBUF round-trips serialized PE/DVE/ACT. v3 removes all of
it:
  - X is host-permuted so an XBAR DMA-transpose (dma_start transpose=True,
    HWDGE) lands xT tiles [128, 8, 128] directly in SBUF.
  - qT is built IN SBUF in a block row layout per 128-row chunk:
    rows [0:32) = u0, [32:64) = u1, [64:96) = ones (memset once,
    buffers are reused), [96:128) = u0*u1. Clamps run on Pool (which
    cannot touch PSUM anyway), the product on DVE. W rows are permuted
    to match, so matmuls consume qT straight from SBUF.
  - PE runs nothing but the 128 block-diagonal matmuls.
  - out is written uint8: the 127x scale is folded into W, evictions add
    +128.5 (psum in [-127,127], so exact-to-half-step under RNE), host
    decodes (v - 128.5)/127. Outputs lie in [-1,1] (bilinear interp of
    +-1 corners); quantization rel err ~3e-3 against the 2e-2 gate.
  - PSUM f32 -> u8 evictions are the remaining elementwise wall
    (~8.4M elem/core); they are split DVE/ACT (the only PSUM-capable
    engines), two PSUM banks per instruction.
  - input DMAs ride the SP HWDGE ring, output DMAs the ACT ring.
"""

import numpy as np
import ml_dtypes
from contextlib import ExitStack

import bass_rust
import concourse.bass as bass
import concourse.mybir as mybir
import concourse.tile as tile
from concourse.bass_utils import run_bass_kernel_spmd

BATCH = 8192
GROUPS = 512
ARITY = 2
OUT_DIM = 16
N_CORES = 8
B_LOC = BATCH // N_CORES          # 1024 rows per core
P = 128                           # partition tile
N_TILES = B_LOC // P              # 8 batch tiles per core
GPC = 32                          # groups per contraction chunk
N_CHUNKS = GROUPS // GPC          # 16
N_XC = (GROUPS * ARITY) // P      # 8 transposed x chunks per tile
CHUNK_COLS = GPC * OUT_DIM        # 512 output cols per chunk (one PSUM bank)
OUT_SCALE = 127.0                 # folded into W; evict adds 128.5 -> uint8

_BF16 = mybir.dt.bfloat16
_F32 = mybir.dt.float32
_U8 = mybir.dt.uint8


def legalize_waits(nc: bass.Bass, cap: int = 1) -> None:
    """Split instructions carrying more than `cap` semaphore waits.

    Hardware instructions have a fixed number of sync-wait slots and walrus
    rejects overflow ("Too many sync wait commands"). Move the excess onto
    NoOp instructions inserted immediately before on the same engine.
    """
    n = 0
    for f in nc.m.functions:
        for bb in f.blocks:
            insts = bb.instructions
            out = []
            changed = False
            for ins in insts:
                si = ins.sync_info
                if si is not None and len(si.on_wait) > cap:
                    waits = list(si.on_wait)
                    keep, extra = waits[:cap], waits[cap:]
                    while extra:
                        chunk, extra = extra[:cap], extra[cap:]
                        nop = mybir.InstNoOp(name=f"wait-legalize-{n}")
                        n += 1
                        nop.engine = ins.engine
                        nop.sync_info = bass_rust.SyncInfo(
                            on_wait=chunk, on_update=[]
                        )
                        out.append(nop)
                    ins.sync_info = bass_rust.SyncInfo(
                        on_wait=keep, on_update=si.on_update
                    )
                    changed = True
                out.append(ins)
            if changed:
                bb.instructions = out


def build_nc(legalize: bool = True) -> bass.Bass:
    nc = bass.Bass()
    x = nc.declare_dram_parameter("x", [B_LOC, GROUPS * ARITY], _BF16, isOutput=False)
    w = nc.declare_dram_parameter("w", [P, N_CHUNKS * CHUNK_COLS], _BF16, isOutput=False)
    out = nc.declare_dram_parameter("out", [B_LOC, GROUPS * OUT_DIM], _U8, isOutput=True)

    with tile.TileContext(nc) as tc, ExitStack() as ctx:
        singles = ctx.enter_context(tc.tile_pool(name="singles", bufs=1))
        xtp = ctx.enter_context(tc.tile_pool(name="xtp", bufs=3))
        qtp = ctx.enter_context(tc.tile_pool(name="qtp", bufs=3))
        outp = ctx.enter_context(tc.tile_pool(name="outp", bufs=4, space="PSUM"))
        outs = ctx.enter_context(tc.tile_pool(name="outs", bufs=3))

        # first transposed x tile + W on the SP HWDGE ring at t=0
        xt0 = xtp.tile([P, N_XC, P], _BF16, tag="xt")
        nc.sync.dma_start(out=xt0[:], in_=x[0:P, :], transpose=True)
        w_sb = singles.tile([P, N_CHUNKS * CHUNK_COLS], _BF16)
        nc.sync.dma_start(out=w_sb[:], in_=w[:])

        # per-partition bias constant for ACT-engine evictions
        bias_c = singles.tile([P, 1], _F32)
        nc.gpsimd.memset(bias_c[:], 128.5)

        # qT buffers reused even/odd; ones rows written once, never touched
        qt_bufs = [
            qtp.tile([P, N_CHUNKS, P], _BF16, tag=f"qt{i}", name=f"qtbuf{i}")
            for i in range(3)
        ]
        for qt in qt_bufs:
            nc.gpsimd.memset(qt[64:96, :, :], 1.0)
        # u1 duplicated at partition base 0: tensor_tensor requires equal
        # input base partitions, so the u0*u1 product reads u0 from qt[0:32]
        # and u1 from this scratch instead of qt[32:64]
        sc_bufs = [
            singles.tile([GPC, 2, N_XC, P], _BF16, name=f"scbuf{i}")
            for i in range(3)
        ]

        for it in range(N_TILES):
            if it == 0:
                xt = xt0
            else:
                xt = xtp.tile([P, N_XC, P], _BF16, tag="xt")
                nc.sync.dma_start(
                    out=xt[:], in_=x[it * P:(it + 1) * P, :], transpose=True
                )

            qt = qt_bufs[it % 3]
            sc = sc_bufs[it % 3]
            # clamps on Pool (SBUF-only engine), dup-clamps + products on DVE
            nc.gpsimd.tensor_scalar(
                out=qt[0:64, 0::2, :], in0=xt[0:64, :, :],
                scalar1=1.0, scalar2=-1.0,
                op0=mybir.AluOpType.min, op1=mybir.AluOpType.max,
            )
            nc.gpsimd.tensor_scalar(
                out=qt[0:64, 1::2, :], in0=xt[64:128, :, :],
                scalar1=1.0, scalar2=-1.0,
                op0=mybir.AluOpType.min, op1=mybir.AluOpType.max,
            )
            nc.gpsimd.tensor_scalar(
                out=sc[:, 0, :, :], in0=xt[GPC:2 * GPC, :, :],
                scalar1=1.0, scalar2=-1.0,
                op0=mybir.AluOpType.min, op1=mybir.AluOpType.max,
            )
            nc.gpsimd.tensor_scalar(
                out=sc[:, 1, :, :], in0=xt[3 * GPC:4 * GPC, :, :],
                scalar1=1.0, scalar2=-1.0,
                op0=mybir.AluOpType.min, op1=mybir.AluOpType.max,
            )
            nc.vector.tensor_tensor(
                out=qt[96:128, 0::2, :], in0=qt[0:GPC, 0::2, :],
                in1=sc[:, 0, :, :], op=mybir.AluOpType.mult,
            )
            nc.vector.tensor_tensor(
                out=qt[96:128, 1::2, :], in0=qt[0:GPC, 1::2, :],
                in1=sc[:, 1, :, :], op=mybir.AluOpType.mult,
            )

            out_sb = outs.tile([P, N_CHUNKS * CHUNK_COLS], _U8)
            o_ps = None
            for j in range(N_CHUNKS):
                # two chunks share a [128, 2, 512] psum tile (2 banks);
                # evict both with one instruction
                if j % 2 == 0:
                    o_ps = outp.tile([P, 2, CHUNK_COLS], _F32)
                nc.tensor.matmul(
                    o_ps[:, j % 2, :], lhsT=qt[:, j, :],
                    rhs=w_sb[:, j * CHUNK_COLS:(j + 1) * CHUNK_COLS],
                    start=True, stop=True,
                )
                if j % 2 == 1:
                    p_idx = j // 2          # 0..7
                    dst = out_sb[:, (j - 1) * CHUNK_COLS:(j + 1) * CHUNK_COLS]
                    src = o_ps[:].rearrange("p k c -> p (k c)")
                    if (1, 0, 1, 1, 0, 1, 1, 0)[p_idx]:
                        nc.scalar.activation(
                            dst, src, mybir.ActivationFunctionType.Identity,
                            bias=bias_c[:], scale=1.0,
                        )
                    else:
                        nc.vector.tensor_scalar_add(dst, src, 128.5)

            # one contiguous 1 MiB u8 output DMA per tile on the ACT ring
            nc.scalar.dma_start(
                out=out[it * P:(it + 1) * P, :], in_=out_sb[:]
            )
    if legalize:
        legalize_waits(nc)
    return nc


def make_w_host(params: np.ndarray) -> np.ndarray:
    """Block-row coefficient matrix (matches the qT row layout):
    w_host[p, t*512 + gl*16 + d] = csel[p//32][32t + gl, d] * OUT_SCALE
    for gl = p % 32, csel = [c1(u0), c2(u1), c0(ones), c3(u0*u1)]."""
    p4 = np.asarray(params, dtype=np.float32)            # [G, 4, D]
    p00, p01, p10, p11 = p4[:, 0], p4[:, 1], p4[:, 2], p4[:, 3]
    c0 = (p00 + p01 + p10 + p11) * 0.25
    c1 = (p10 + p11 - p00 - p01) * 0.25
    c2 = (p01 + p11 - p00 - p10) * 0.25
    c3 = (p00 + p11 - p01 - p10) * 0.25
    csel = [c1, c2, c0, c3]                              # row-block order
    wm = np.zeros((N_CHUNKS, P, CHUNK_COLS), np.float32)
    for ci in range(4):
        cs = csel[ci].reshape(N_CHUNKS, GPC, OUT_DIM) * OUT_SCALE
        for gl in range(GPC):
            wm[:, 32 * ci + gl, gl * OUT_DIM:(gl + 1) * OUT_DIM] = cs[:, gl]
    w_host = np.ascontiguousarray(wm.transpose(1, 0, 2).reshape(P, N_CHUNKS * CHUNK_COLS))
    return w_host.astype(ml_dtypes.bfloat16)


_NC_CACHE = {}


def make_in_maps(X: np.ndarray, params: np.ndarray) -> list[dict]:
    X = np.asarray(X, dtype=np.float32)
    assert X.shape == (BATCH, GROUPS * ARITY)
    # host column permutation so the DMA-transposed tile has, per 128-col
    # chunk c: rows [0:32) x0 / [32:64) x1 of groups 64c..64c+31, rows
    # [64:96) x0 / [96:128) x1 of groups 64c+32..64c+63.
    # col = 128c + 64*half + 32*a + gl  <->  g = 64c + 32*half + gl
    Xp = (
        X.reshape(BATCH, N_XC, 2, GPC, ARITY)
        .transpose(0, 1, 2, 4, 3)
        .reshape(BATCH, GROUPS * ARITY)
    )
    X16 = np.ascontiguousarray(Xp.astype(ml_dtypes.bfloat16))
    w_host = make_w_host(params)
    return [
        {"x": X16[i * B_LOC:(i + 1) * B_LOC], "w": w_host} for i in range(N_CORES)
    ]


def kernel(X: np.ndarray, params: np.ndarray) -> np.ndarray:
    in_maps = make_in_maps(X, params)

    if "nc" not in _NC_CACHE:
        _NC_CACHE["nc"] = build_nc()
    nc = _NC_CACHE["nc"]

    res = run_bass_kernel_spmd(nc, in_maps, core_ids=list(range(N_CORES)))
    out_u8 = np.concatenate(
        [np.asarray(res.results[i]["out"]) for i in range(N_CORES)], axis=0
    )
    return decode_out(out_u8)


def decode_out(out_u8: np.ndarray) -> np.ndarray:
    # inverse of the on-device encode round(127*x + 128.5)
    return (out_u8.astype(np.float32) - 128.5) * (1.0 / OUT_SCALE)


# revision 15
# speedup vs baseline: 1.0528x; 1.0528x over previous
"""Trainium2 Bass kernel for nn_CornerActivationB.

Math: the reference expands a binary corner table [G, 4, D] to a ternary
grid [G, 9, D] via midpoint averaging, then does piecewise-bilinear
interpolation on the 3x3 grid. Midpoints are exact averages, so the
piecewise-bilinear interpolant of those samples IS the bilinear function
of the 4 binary corners:

    out[b, g, d] = c0[g,d] + u0*c1[g,d] + u1*c2[g,d] + u0*u1*c3[g,d]

with u = clip(x, -1, 1) and c* fixed +-0.25-multiples of corner sums.

v3 (transpose-free): the baseline was DMA-limited; v2 (uint8 out, bf16
in) fixed that but left the kernel dependency-bound: 123 PE transposes
plus qT PSUM->SBUF round-trips serialized PE/DVE/ACT. v3 removes all of
it:
  - X is host-permuted so an XBAR DMA-transpose (dma_start transpose=True,
    HWDGE) lands xT tiles [128, 8, 128] directly in SBUF.
  - qT is built IN SBUF in a block row layout per 128-row chunk:
    rows [0:32) = u0, [32:64) = u1, [64:96) = ones (memset once,
    buffers are reused), [96:128) = u0*u1. Clamps run on Pool (which
    cannot touch PSUM anyway), the product on DVE. W rows are permuted
    to match, so matmuls consume qT straight from SBUF.
  - PE runs nothing but the 128 block-diagonal matmuls.
  - out is written uint8: the 127x scale is folded into W, evictions add
    +128.5 (psum in [-127,127], so exact-to-half-step under RNE), host
    decodes (v - 128.5)/127. Outputs lie in [-1,1] (bilinear interp of
    +-1 corners); quantization rel err ~3e-3 against the 2e-2 gate.
  - PSUM f32 -> u8 evictions are the remaining elementwise wall
    (~8.4M elem/core); they are split DVE/ACT (the only PSUM-capable
    engines), two PSUM banks per instruction.
  - input DMAs ride the SP HWDGE ring, output DMAs the ACT ring.
"""

import numpy as np
import ml_dtypes
from contextlib import ExitStack

import bass_rust
import concourse.bass as bass
import concourse.mybir as mybir
import concourse.tile as tile
from concourse.bass_utils import run_bass_kernel_spmd

BATCH = 8192
GROUPS = 512
ARITY = 2
OUT_DIM = 16
N_CORES = 8
B_LOC = BATCH // N_CORES          # 1024 rows per core
P = 128                           # partition tile
N_TILES = B_LOC // P              # 8 batch tiles per core
GPC = 32                          # groups per contraction chunk
N_CHUNKS = GROUPS // GPC          # 16
N_XC = (GROUPS * ARITY) // P      # 8 transposed x chunks per tile
CHUNK_COLS = GPC * OUT_DIM        # 512 output cols per chunk (one PSUM bank)
OUT_SCALE = 127.0                 # folded into W; evict adds 128.5 -> uint8

_BF16 = mybir.dt.bfloat16
_F32 = mybir.dt.float32
_U8 = mybir.dt.uint8


def legalize_waits(nc: bass.Bass, cap: int = 1) -> None:
    """Split instructions carrying more than `cap` semaphore waits.

    Hardware instructions have a fixed number of sync-wait slots and walrus
    rejects overflow ("Too many sync wait commands"). Move the excess onto
    NoOp instructions inserted immediately before on the same engine.
    """
    n = 0
    for f in nc.m.functions:
        for bb in f.blocks:
            insts = bb.instructions
            out = []
            changed = False
            for ins in insts:
                si = ins.sync_info
                if si is not None and len(si.on_wait) > cap:
                    waits = list(si.on_wait)
                    keep, extra = waits[:cap], waits[cap:]
                    while extra:
                        chunk, extra = extra[:cap], extra[cap:]
                        nop = mybir.InstNoOp(name=f"wait-legalize-{n}")
                        n += 1
                        nop.engine = ins.engine
                        nop.sync_info = bass_rust.SyncInfo(
                            on_wait=chunk, on_update=[]
                        )
                        out.append(nop)
                    ins.sync_info = bass_rust.SyncInfo(
                        on_wait=keep, on_update=si.on_update
                    )
                    changed = True
                out.append(ins)
            if changed:
                bb.instructions = out


def build_nc(legalize: bool = True) -> bass.Bass:
    nc = bass.Bass()
    x = nc.declare_dram_parameter("x", [B_LOC, GROUPS * ARITY], _BF16, isOutput=False)
    w = nc.declare_dram_parameter("w", [P, N_CHUNKS * CHUNK_COLS], _BF16, isOutput=False)
    out = nc.declare_dram_parameter("out", [B_LOC, GROUPS * OUT_DIM], _U8, isOutput=True)

    with tile.TileContext(nc) as tc, ExitStack() as ctx:
        singles = ctx.enter_context(tc.tile_pool(name="singles", bufs=1))
        xtp = ctx.enter_context(tc.tile_pool(name="xtp", bufs=3))
        qtp = ctx.enter_context(tc.tile_pool(name="qtp", bufs=3))
        outp = ctx.enter_context(tc.tile_pool(name="outp", bufs=4, space="PSUM"))
        outs = ctx.enter_context(tc.tile_pool(name="outs", bufs=3))

        # first transposed x tile + W on the SP HWDGE ring at t=0
        xt0 = xtp.tile([P, N_XC, P], _BF16, tag="xt")
        nc.sync.dma_start(out=xt0[:], in_=x[0:P, :], transpose=True)
        w_sb = singles.tile([P, N_CHUNKS * CHUNK_COLS], _BF16)
        nc.sync.dma_start(out=w_sb[:], in_=w[:])

        # per-partition bias constant for ACT-engine evictions
        bias_c = singles.tile([P, 1], _F32)
        nc.gpsimd.memset(bias_c[:], 128.5)

        # qT buffers reused even/odd; ones rows written once, never touched
        qt_bufs = [
            qtp.tile([P, N_CHUNKS, P], _BF16, tag=f"qt{i}", name=f"qtbuf{i}")
            for i in range(3)
        ]
        for qt in qt_bufs:
            nc.gpsimd.memset(qt[64:96, :, :], 1.0)
        # u1 duplicated at partition base 0: tensor_tensor requires equal
        # input base partitions, so the u0*u1 product reads u0 from qt[0:32]
        # and u1 from this scratch instead of qt[32:64]
        sc_bufs = [
            singles.tile([GPC, 2, N_XC, P], _BF16, name=f"scbuf{i}")
            for i in range(3)
        ]

        for it in range(N_TILES):
            if it == 0:
                xt = xt0
            else:
                xt = xtp.tile([P, N_XC, P], _BF16, tag="xt")
                nc.sync.dma_start(
                    out=xt[:], in_=x[it * P:(it + 1) * P, :], transpose=True
                )

            qt = qt_bufs[it % 3]
            sc = sc_bufs[it % 3]
            # clamps on Pool (SBUF-only engine), dup-clamps + products on DVE
            nc.gpsimd.tensor_scalar(
                out=qt[0:64, 0::2, :], in0=xt[0:64, :, :],
                scalar1=1.0, scalar2=-1.0,
                op0=mybir.AluOpType.min, op1=mybir.AluOpType.max,
            )
            nc.gpsimd.tensor_scalar(
                out=qt[0:64, 1::2, :], in0=xt[64:128, :, :],
                scalar1=1.0, scalar2=-1.0,
                op0=mybir.AluOpType.min, op1=mybir.AluOpType.max,
            )
            nc.vector.tensor_scalar(
                out=sc[:, 0, :, :], in0=xt[GPC:2 * GPC, :, :],
                scalar1=1.0, scalar2=-1.0,
                op0=mybir.AluOpType.min, op1=mybir.AluOpType.max,
            )
            nc.vector.tensor_scalar(
                out=sc[:, 1, :, :], in0=xt[3 * GPC:4 * GPC, :, :],
                scalar1=1.0, scalar2=-1.0,
                op0=mybir.AluOpType.min, op1=mybir.AluOpType.max,
            )
            nc.vector.tensor_tensor(
                out=qt[96:128, 0::2, :], in0=qt[0:GPC, 0::2, :],
                in1=sc[:, 0, :, :], op=mybir.AluOpType.mult,
            )
            nc.vector.tensor_tensor(
                out=qt[96:128, 1::2, :], in0=qt[0:GPC, 1::2, :],
                in1=sc[:, 1, :, :], op=mybir.AluOpType.mult,
            )

            out_sb = outs.tile([P, N_CHUNKS * CHUNK_COLS], _U8)
            o_ps = None
            for j in range(N_CHUNKS):
                # two chunks share a [128, 2, 512] psum tile (2 banks);
                # evict both with one instruction
                if j % 2 == 0:
                    o_ps = outp.tile([P, 2, CHUNK_COLS], _F32)
                nc.tensor.matmul(
                    o_ps[:, j % 2, :], lhsT=qt[:, j, :],
                    rhs=w_sb[:, j * CHUNK_COLS:(j + 1) * CHUNK_COLS],
                    start=True, stop=True,
                )
                if j % 2 == 1:
                    p_idx = j // 2          # 0..7
                    dst = out_sb[:, (j - 1) * CHUNK_COLS:(j + 1) * CHUNK_COLS]
                    src = o_ps[:].rearrange("p k c -> p (k c)")
                    if (1, 0, 1, 1, 0, 1, 1, 0)[p_idx]:
                        nc.scalar.activation(
                            dst, src, mybir.ActivationFunctionType.Identity,
                            bias=bias_c[:], scale=1.0,
                        )
                    else:
                        nc.vector.tensor_scalar_add(dst, src, 128.5)

            # one contiguous 1 MiB u8 output DMA per tile on the ACT ring
            nc.scalar.dma_start(
                out=out[it * P:(it + 1) * P, :], in_=out_sb[:]
            )
    if legalize:
        legalize_waits(nc)
    return nc


def make_w_host(params: np.ndarray) -> np.ndarray:
    """Block-row coefficient matrix (matches the qT row layout):
    w_host[p, t*512 + gl*16 + d] = csel[p//32][32t + gl, d] * OUT_SCALE
    for gl = p % 32, csel = [c1(u0), c2(u1), c0(ones), c3(u0*u1)]."""
    p4 = np.asarray(params, dtype=np.float32)            # [G, 4, D]
    p00, p01, p10, p11 = p4[:, 0], p4[:, 1], p4[:, 2], p4[:, 3]
    c0 = (p00 + p01 + p10 + p11) * 0.25
    c1 = (p10 + p11 - p00 - p01) * 0.25
    c2 = (p01 + p11 - p00 - p10) * 0.25
    c3 = (p00 + p11 - p01 - p10) * 0.25
    csel = [c1, c2, c0, c3]                              # row-block order
    wm = np.zeros((N_CHUNKS, P, CHUNK_COLS), np.float32)
    for ci in range(4):
        cs = csel[ci].reshape(N_CHUNKS, GPC, OUT_DIM) * OUT_SCALE
        for gl in range(GPC):
            wm[:, 32 * ci + gl, gl * OUT_DIM:(gl + 1) * OUT_DIM] = cs[:, gl]
    w_host = np.ascontiguousarray(wm.transpose(1, 0, 2).reshape(P, N_CHUNKS * CHUNK_COLS))
    return w_host.astype(ml_dtypes.bfloat16)


_NC_CACHE = {}


def make_in_maps(X: np.ndarray, params: np.ndarray) -> list[dict]:
    X = np.asarray(X, dtype=np.float32)
    assert X.shape == (BATCH, GROUPS * ARITY)
    # host column permutation so the DMA-transposed tile has, per 128-col
    # chunk c: rows [0:32) x0 / [32:64) x1 of groups 64c..64c+31, rows
    # [64:96) x0 / [96:128) x1 of groups 64c+32..64c+63.
    # col = 128c + 64*half + 32*a + gl  <->  g = 64c + 32*half + gl
    Xp = (
        X.reshape(BATCH, N_XC, 2, GPC, ARITY)
        .transpose(0, 1, 2, 4, 3)
        .reshape(BATCH, GROUPS * ARITY)
    )
    X16 = np.ascontiguousarray(Xp.astype(ml_dtypes.bfloat16))
    w_host = make_w_host(params)
    return [
        {"x": X16[i * B_LOC:(i + 1) * B_LOC], "w": w_host} for i in range(N_CORES)
    ]


def kernel(X: np.ndarray, params: np.ndarray) -> np.ndarray:
    in_maps = make_in_maps(X, params)

    if "nc" not in _NC_CACHE:
        _NC_CACHE["nc"] = build_nc()
    nc = _NC_CACHE["nc"]

    res = run_bass_kernel_spmd(nc, in_maps, core_ids=list(range(N_CORES)))
    out_u8 = np.concatenate(
        [np.asarray(res.results[i]["out"]) for i in range(N_CORES)], axis=0
    )
    return decode_out(out_u8)


def decode_out(out_u8: np.ndarray) -> np.ndarray:
    # inverse of the on-device encode round(127*x + 128.5)
    return (out_u8.astype(np.float32) - 128.5) * (1.0 / OUT_SCALE)
